# revision 5
# baseline (speedup 1.0000x reference)
"""Single-head causal attention (B=4, L=2048, D=1024) on 8 TRN2 NeuronCores.

Sharding: data-parallel over batch (4) x query-halves (2) = 8 cores.
Each core computes K/V projections over its batch's full 2048 keys and
Q/scores/AV over its 1024-query half.

Device-side layout (per core, all bf16 matmul operands, f32 PSUM accum):
  - scores computed TRANSPOSED: sT[k_tile(128 part), q(1024 free)] so the
    pad mask (a per-key quantity) is a per-partition tensor_scalar and the
    softmax normalizer Z comes from a ones-column appended to V (no
    partition reductions, no P transposes anywhere).
  - masked_fill semantics (exact, incl. degenerate all-masked rows):
      cmp[k,q]  = (iota_q >= thresh[k]) * padkeep[k]   in {0,1}
      sM        = (s_raw + 960) * cmp                  # 0 where masked
      E         = exp(sM/32 - 30)                      # = exp(s/32) kept,
                                                       #   exp(-30) masked
    All-masked rows get exactly-uniform weights, matching jax softmax of
    an all -10000 row.
  - out[q,m] = (E^T V)[q,m] / Z[q], Z from the ones column of V.
"""
import sys

if "/opt/trn_rl_repo" not in sys.path:
    sys.path.insert(0, "/opt/trn_rl_repo")

import numpy as np
import ml_dtypes

import concourse.bass as bass
import concourse.mybir as mybir
from concourse import bacc, tile
from concourse import bass_utils

F32 = mybir.dt.float32
BF16 = mybir.dt.bfloat16
BF16NP = ml_dtypes.bfloat16

B, L, D = 4, 2048, 1024
NQ = L // 2          # queries per core
NKT = L // 128       # 16 k-tiles
NMC = D // 128       # 8 contraction/model chunks
NQT = NQ // 128      # 8 q-tiles per core

_NC_CACHE = None


def _build_nc():
    nc = bacc.Bacc(None, target_bir_lowering=False)

    xt_d = nc.dram_tensor("xt", [128, NMC, L], BF16, kind="ExternalInput")
    xq_d = nc.dram_tensor("xq", [128, NMC, NQ], BF16, kind="ExternalInput")
    wq_d = nc.dram_tensor("wq", [128, NMC, D], BF16, kind="ExternalInput")
    wk_d = nc.dram_tensor("wk", [128, NMC, D], BF16, kind="ExternalInput")
    wv_d = nc.dram_tensor("wv", [128, NMC, D], BF16, kind="ExternalInput")
    padk_d = nc.dram_tensor("padk", [128, NKT], F32, kind="ExternalInput")
    thr_d = nc.dram_tensor("thr", [128, NKT], F32, kind="ExternalInput")
    out_d = nc.dram_tensor("out", [NQ, D], F32, kind="ExternalOutput")

    AL = mybir.AluOpType
    AF = mybir.ActivationFunctionType

    with tile.TileContext(nc) as tc:
        with (
            tc.tile_pool(name="c", bufs=1) as cpool,
            tc.tile_pool(name="sh", bufs=1) as spool,
            tc.tile_pool(name="wk_", bufs=3) as wpool,
            tc.tile_pool(name="pp", bufs=3, space="PSUM") as pp,
            tc.tile_pool(name="ppo", bufs=2, space="PSUM") as ppo,
            tc.tile_pool(name="ppz", bufs=1, space="PSUM") as ppz,
        ):
            # persistent tiles
            xt_sb = spool.tile([128, NMC, L], BF16, name="xt_sb", tag="big")
            wk_sb = spool.tile([128, NMC, D], BF16, name="wk_sb", tag="wkqt")
            wv_sb = cpool.tile([128, NMC, D], BF16, name="wv_sb")
            wq_sb = cpool.tile([128, NMC, D], BF16, name="wq_sb")
            xq_sb = cpool.tile([128, NMC, NQ], BF16, name="xq_sb")
            padk_sb = cpool.tile([128, NKT], F32, name="padk_sb")
            thr_sb = cpool.tile([128, NKT], F32, name="thr_sb")
            kT_sb = cpool.tile([128, NMC, L], BF16, name="kT_sb")
            v_sb = cpool.tile([128, NKT, D + 1], BF16, name="v_sb")
            iota_sb = cpool.tile([128, NQ], F32, name="iota_sb")
            bias_sb = cpool.tile([128, 1], F32, name="bias_sb")

            nc.sync.dma_start(xt_sb[:], xt_d[:])
            nc.sync.dma_start(wk_sb[:], wk_d[:])
            nc.sync.dma_start(wv_sb[:], wv_d[:])
            nc.sync.dma_start(wq_sb[:], wq_d[:])
            nc.sync.dma_start(xq_sb[:], xq_d[:])
            nc.sync.dma_start(padk_sb[:], padk_d[:])
            nc.sync.dma_start(thr_sb[:], thr_d[:])

            nc.gpsimd.iota(
                out=iota_sb[:], pattern=[[1, NQ]], base=0, channel_multiplier=0,
                allow_small_or_imprecise_dtypes=True,
            )
            nc.vector.memset(bias_sb[:], -30.0)
            nc.vector.memset(v_sb[:, :, D : D + 1], 1.0)

            # ---- Phase 1a: kT[m, tok] = wk.T @ x ----
            for mi in range(NMC):
                for tb in range(L // 512):
                    ps = pp.tile([128, 512], F32, name="ps")
                    for d in range(NMC):
                        nc.tensor.matmul(
                            ps[:],
                            lhsT=wk_sb[:, d, mi * 128 : (mi + 1) * 128],
                            rhs=xt_sb[:, d, tb * 512 : (tb + 1) * 512],
                            start=(d == 0), stop=(d == NMC - 1),
                        )
                    nc.scalar.copy(kT_sb[:, mi, tb * 512 : (tb + 1) * 512], ps[:])

            # ---- Phase 1b: V[tok, m] = x @ wv  (lhsT = xT chunk) ----
            for kt in range(NKT):
                for mb in range(D // 512):
                    ps = pp.tile([128, 512], F32, name="ps")
                    for d in range(NMC):
                        nc.tensor.matmul(
                            ps[:],
                            lhsT=xt_sb[:, d, kt * 128 : (kt + 1) * 128],
                            rhs=wv_sb[:, d, mb * 512 : (mb + 1) * 512],
                            start=(d == 0), stop=(d == NMC - 1),
                        )
                    nc.scalar.copy(v_sb[:, kt, mb * 512 : (mb + 1) * 512], ps[:])

            # ---- Phase 1c: qT[m, q] = wq.T @ xq ----
            qT_sb = spool.tile([128, NMC, NQ], BF16, name="qT_sb", tag="wkqt")
            for mi in range(NMC):
                for qb in range(NQ // 512):
                    ps = pp.tile([128, 512], F32, name="ps")
                    for d in range(NMC):
                        nc.tensor.matmul(
                            ps[:],
                            lhsT=wq_sb[:, d, mi * 128 : (mi + 1) * 128],
                            rhs=xq_sb[:, d, qb * 512 : (qb + 1) * 512],
                            start=(d == 0), stop=(d == NMC - 1),
                        )
                    nc.scalar.copy(qT_sb[:, mi, qb * 512 : (qb + 1) * 512], ps[:])

            # ---- Phase 2: scores (transposed) + mask + exp, per k-tile ----
            E_sb = spool.tile([128, NKT, NQ], BF16, name="E_sb", tag="big")
            for kt in range(NKT):
                cmp = wpool.tile([128, NQ], F32, name="cmp", bufs=2)
                nc.gpsimd.tensor_scalar(
                    out=cmp[:], in0=iota_sb[:],
                    scalar1=thr_sb[:, kt : kt + 1], scalar2=padk_sb[:, kt : kt + 1],
                    op0=AL.is_ge, op1=AL.mult,
                )
                s_sb = wpool.tile([128, NQ], F32, name="s_sb", bufs=3)
                for qb in range(NQ // 512):
                    ps = pp.tile([128, 512], F32, name="ps")
                    for m in range(NMC):
                        nc.tensor.matmul(
                            ps[:],
                            lhsT=kT_sb[:, m, kt * 128 : (kt + 1) * 128],
                            rhs=qT_sb[:, m, qb * 512 : (qb + 1) * 512],
                            start=(m == 0), stop=(m == NMC - 1),
                        )
                    nc.vector.scalar_tensor_tensor(
                        out=s_sb[:, qb * 512 : (qb + 1) * 512], in0=ps[:],
                        scalar=960.0, in1=cmp[:, qb * 512 : (qb + 1) * 512],
                        op0=AL.add, op1=AL.mult,
                    )
                nc.scalar.activation(
                    out=E_sb[:, kt, :], in_=s_sb[:],
                    func=AF.Exp, bias=bias_sb[:], scale=0.03125,
                )

            # ---- Phase 3: out[q,m] = (E^T @ [V|1])[q,m] / Z[q] ----
            for qt in range(NQT):
                po = ppo.tile([128, D], F32, name="po")
                pz = ppz.tile([128, 1], F32, name="pz")
                for kt in range(NKT):
                    lhsT = E_sb[:, kt, qt * 128 : (qt + 1) * 128]
                    nc.tensor.matmul(po[:, 0:512], lhsT=lhsT, rhs=v_sb[:, kt, 0:512],
                                     start=(kt == 0), stop=(kt == NKT - 1))
                    nc.tensor.matmul(po[:, 512:1024], lhsT=lhsT, rhs=v_sb[:, kt, 512:1024],
                                     start=(kt == 0), stop=(kt == NKT - 1))
                    nc.tensor.matmul(pz[:], lhsT=lhsT, rhs=v_sb[:, kt, D : D + 1],
                                     start=(kt == 0), stop=(kt == NKT - 1))
                rec = wpool.tile([128, 1], F32, name="rec", bufs=2)
                nc.vector.reciprocal(rec[:], pz[:])
                o_sb = wpool.tile([128, D], F32, name="o_sb", bufs=3)
                nc.vector.tensor_scalar(
                    out=o_sb[:], in0=po[:], scalar1=rec[:], scalar2=None, op0=AL.mult,
                )
                nc.sync.dma_start(out_d[qt * 128 : (qt + 1) * 128, :], o_sb[:])

    nc.compile()
    return nc


def _chunked(a):
    """[C*128, N] -> [128, C, N] contiguous."""
    c = a.shape[0] // 128
    return np.ascontiguousarray(a.reshape(c, 128, *a.shape[1:]).transpose(1, 0, 2))


def build_in_maps(inputs):
    x = np.asarray(inputs["x"], dtype=np.float32)
    pad = np.asarray(inputs["pad_mask"])
    wq_h = _chunked(np.asarray(inputs["wq"], dtype=np.float32)).astype(BF16NP)
    wk_h = _chunked(np.asarray(inputs["wk"], dtype=np.float32)).astype(BF16NP)
    wv_h = _chunked(np.asarray(inputs["wv"], dtype=np.float32)).astype(BF16NP)

    in_maps = []
    for c in range(8):
        b, h = divmod(c, 2)
        q0 = h * NQ
        xtb = _chunked(x[b].T).astype(BF16NP)               # [128, 8, 2048]
        xqb = _chunked(x[b, q0 : q0 + NQ, :].T).astype(BF16NP)  # [128, 8, 1024]
        keep = (~pad[b]).astype(np.float32)                     # [2048]
        padk = np.ascontiguousarray(keep.reshape(NKT, 128).T)   # [128, 16]
        thr = (
            np.add.outer(np.arange(128, dtype=np.float32),
                         128.0 * np.arange(NKT, dtype=np.float32))
            - np.float32(q0)
        ).astype(np.float32)                                    # [128, 16]
        in_maps.append({
            "xt": xtb, "xq": xqb, "wq": wq_h, "wk": wk_h, "wv": wv_h,
            "padk": padk, "thr": np.ascontiguousarray(thr),
        })
    return in_maps


def kernel(**inputs):
    global _NC_CACHE
    if _NC_CACHE is None:
        _NC_CACHE = _build_nc()
    nc = _NC_CACHE

    in_maps = build_in_maps(inputs)
    res = bass_utils.run_bass_kernel_spmd(nc, in_maps, core_ids=list(range(8)))
    out = np.stack(
        [np.concatenate([res.results[2 * b]["out"], res.results[2 * b + 1]["out"]], axis=0)
         for b in range(B)]
    )
    return out.astype(np.float32)


# revision 6
# speedup vs baseline: 1.5828x; 1.5828x over previous
"""Single-head causal attention (B=4, L=2048, D=1024) on 8 TRN2 NeuronCores.

Sharding: data-parallel over batch (4) x query-halves (2) = 8 cores.
Each core computes K/V projections over its batch's full 2048 keys and
Q/scores/AV over its 1024-query half.

Device-side layout (per core, all bf16 matmul operands, f32 PSUM accum):
  - scores computed TRANSPOSED: sT[k_tile(128 part), q(1024 free)] so the
    pad mask (a per-key quantity) is a per-partition tensor_scalar and the
    softmax normalizer Z comes from a ones-column appended to V (no
    partition reductions, no P transposes anywhere).
  - masked_fill semantics (exact, incl. degenerate all-masked rows):
      cmp[k,q]  = (iota_q >= thresh[k]) * padkeep[k]   in {0,1}
      sM        = (s_raw + 960) * cmp                  # 0 where masked
      E         = exp(sM/32 - 30)                      # = exp(s/32) kept,
                                                       #   exp(-30) masked
    All-masked rows get exactly-uniform weights, matching jax softmax of
    an all -10000 row.
  - out[q,m] = (E^T V)[q,m] / Z[q], Z from the ones column of V.
"""
import sys

if "/opt/trn_rl_repo" not in sys.path:
    sys.path.insert(0, "/opt/trn_rl_repo")

import numpy as np
import ml_dtypes

import concourse.bass as bass
import concourse.mybir as mybir
from concourse import bacc, tile
from concourse import bass_utils

F32 = mybir.dt.float32
BF16 = mybir.dt.bfloat16
BF16NP = ml_dtypes.bfloat16

B, L, D = 4, 2048, 1024
NQ = L // 2          # queries per core
NKT = L // 128       # 16 k-tiles
NMC = D // 128       # 8 contraction/model chunks
NQT = NQ // 128      # 8 q-tiles per core

_NC_CACHE = None


def _build_nc():
    nc = bacc.Bacc(None, target_bir_lowering=False)

    xt_d = nc.dram_tensor("xt", [128, NMC, L], BF16, kind="ExternalInput")
    xq_d = nc.dram_tensor("xq", [128, NMC, NQ], BF16, kind="ExternalInput")
    wq_d = nc.dram_tensor("wq", [128, NMC, D], BF16, kind="ExternalInput")
    wk_d = nc.dram_tensor("wk", [128, NMC, D], BF16, kind="ExternalInput")
    wv_d = nc.dram_tensor("wv", [128, NMC, D], BF16, kind="ExternalInput")
    padk_d = nc.dram_tensor("padk", [128, NKT], F32, kind="ExternalInput")
    thr_d = nc.dram_tensor("thr", [128, NKT], F32, kind="ExternalInput")
    out_d = nc.dram_tensor("out", [NQ, D], F32, kind="ExternalOutput")

    AL = mybir.AluOpType
    AF = mybir.ActivationFunctionType

    with tile.TileContext(nc) as tc:
        with (
            tc.tile_pool(name="c", bufs=1) as cpool,
            tc.tile_pool(name="sh", bufs=1) as spool,
            tc.tile_pool(name="wk_", bufs=3) as wpool,
            tc.tile_pool(name="pp", bufs=3, space="PSUM") as pp,
            tc.tile_pool(name="ppo", bufs=2, space="PSUM") as ppo,
            tc.tile_pool(name="ppz", bufs=1, space="PSUM") as ppz,
        ):
            # persistent tiles
            xt_sb = spool.tile([128, NMC, L], BF16, name="xt_sb", tag="big")
            wk_sb = spool.tile([128, NMC, D], BF16, name="wk_sb", tag="wkqt")
            wv_sb = cpool.tile([128, NMC, D], BF16, name="wv_sb")
            wq_sb = cpool.tile([128, NMC, D], BF16, name="wq_sb")
            xq_sb = cpool.tile([128, NMC, NQ], BF16, name="xq_sb")
            padk_sb = cpool.tile([128, NKT], F32, name="padk_sb")
            thr_sb = cpool.tile([128, NKT], F32, name="thr_sb")
            kT_sb = cpool.tile([128, NMC, L], BF16, name="kT_sb")
            v_sb = cpool.tile([128, NKT, D + 1], BF16, name="v_sb")
            iota_sb = cpool.tile([128, NQ], F32, name="iota_sb")
            bias_sb = cpool.tile([128, 1], F32, name="bias_sb")

            nc.sync.dma_start(xt_sb[:], xt_d[:])
            nc.sync.dma_start(wk_sb[:], wk_d[:])
            nc.sync.dma_start(wv_sb[:], wv_d[:])
            nc.sync.dma_start(wq_sb[:], wq_d[:])
            nc.sync.dma_start(xq_sb[:], xq_d[:])
            nc.sync.dma_start(padk_sb[:], padk_d[:])
            nc.sync.dma_start(thr_sb[:], thr_d[:])

            nc.gpsimd.iota(
                out=iota_sb[:], pattern=[[1, NQ]], base=0, channel_multiplier=0,
                allow_small_or_imprecise_dtypes=True,
            )
            nc.vector.memset(bias_sb[:], -30.0)
            nc.vector.memset(v_sb[:, :, D : D + 1], 1.0)

            # ---- Phase 1a: kT[m, tok] = wk.T @ x ----
            for mi in range(NMC):
                for tb in range(L // 512):
                    ps = pp.tile([128, 512], F32, name="ps")
                    for d in range(NMC):
                        nc.tensor.matmul(
                            ps[:],
                            lhsT=wk_sb[:, d, mi * 128 : (mi + 1) * 128],
                            rhs=xt_sb[:, d, tb * 512 : (tb + 1) * 512],
                            start=(d == 0), stop=(d == NMC - 1),
                        )
                    nc.scalar.copy(kT_sb[:, mi, tb * 512 : (tb + 1) * 512], ps[:])

            # ---- Phase 1b: V[tok, m] = x @ wv  (lhsT = xT chunk) ----
            for kt in range(NKT):
                for mb in range(D // 512):
                    ps = pp.tile([128, 512], F32, name="ps")
                    for d in range(NMC):
                        nc.tensor.matmul(
                            ps[:],
                            lhsT=xt_sb[:, d, kt * 128 : (kt + 1) * 128],
                            rhs=wv_sb[:, d, mb * 512 : (mb + 1) * 512],
                            start=(d == 0), stop=(d == NMC - 1),
                        )
                    nc.scalar.copy(v_sb[:, kt, mb * 512 : (mb + 1) * 512], ps[:])

            # ---- Phase 1c: qT[m, q] = wq.T @ xq ----
            qT_sb = spool.tile([128, NMC, NQ], BF16, name="qT_sb", tag="wkqt")
            for mi in range(NMC):
                for qb in range(NQ // 512):
                    ps = pp.tile([128, 512], F32, name="ps")
                    for d in range(NMC):
                        nc.tensor.matmul(
                            ps[:],
                            lhsT=wq_sb[:, d, mi * 128 : (mi + 1) * 128],
                            rhs=xq_sb[:, d, qb * 512 : (qb + 1) * 512],
                            start=(d == 0), stop=(d == NMC - 1),
                        )
                    nc.scalar.copy(qT_sb[:, mi, qb * 512 : (qb + 1) * 512], ps[:])

            # ---- Phase 2: scores (transposed) + mask + exp, per k-tile ----
            E_sb = spool.tile([128, NKT, NQ], BF16, name="E_sb", tag="big")
            for kt in range(NKT):
                cmp = wpool.tile([128, NQ], F32, name="cmp", bufs=2)
                nc.vector.tensor_scalar(
                    out=cmp[:], in0=iota_sb[:],
                    scalar1=thr_sb[:, kt : kt + 1], scalar2=padk_sb[:, kt : kt + 1],
                    op0=AL.is_ge, op1=AL.mult,
                )
                s_sb = wpool.tile([128, NQ], F32, name="s_sb", bufs=3)
                for qb in range(NQ // 512):
                    ps = pp.tile([128, 512], F32, name="ps")
                    for m in range(NMC):
                        nc.tensor.matmul(
                            ps[:],
                            lhsT=kT_sb[:, m, kt * 128 : (kt + 1) * 128],
                            rhs=qT_sb[:, m, qb * 512 : (qb + 1) * 512],
                            start=(m == 0), stop=(m == NMC - 1),
                        )
                    nc.vector.scalar_tensor_tensor(
                        out=s_sb[:, qb * 512 : (qb + 1) * 512], in0=ps[:],
                        scalar=960.0, in1=cmp[:, qb * 512 : (qb + 1) * 512],
                        op0=AL.add, op1=AL.mult,
                    )
                nc.scalar.activation(
                    out=E_sb[:, kt, :], in_=s_sb[:],
                    func=AF.Exp, bias=bias_sb[:], scale=0.03125,
                )

            # ---- Phase 3: out[q,m] = (E^T @ [V|1])[q,m] / Z[q] ----
            for qt in range(NQT):
                po = ppo.tile([128, D], F32, name="po")
                pz = ppz.tile([128, 1], F32, name="pz")
                for kt in range(NKT):
                    lhsT = E_sb[:, kt, qt * 128 : (qt + 1) * 128]
                    nc.tensor.matmul(po[:, 0:512], lhsT=lhsT, rhs=v_sb[:, kt, 0:512],
                                     start=(kt == 0), stop=(kt == NKT - 1))
                    nc.tensor.matmul(po[:, 512:1024], lhsT=lhsT, rhs=v_sb[:, kt, 512:1024],
                                     start=(kt == 0), stop=(kt == NKT - 1))
                    nc.tensor.matmul(pz[:], lhsT=lhsT, rhs=v_sb[:, kt, D : D + 1],
                                     start=(kt == 0), stop=(kt == NKT - 1))
                rec = wpool.tile([128, 1], F32, name="rec", bufs=2)
                nc.vector.reciprocal(rec[:], pz[:])
                o_sb = wpool.tile([128, D], F32, name="o_sb", bufs=3)
                nc.vector.tensor_scalar(
                    out=o_sb[:], in0=po[:], scalar1=rec[:], scalar2=None, op0=AL.mult,
                )
                nc.sync.dma_start(out_d[qt * 128 : (qt + 1) * 128, :], o_sb[:])

    nc.compile()
    return nc


def _chunked(a):
    """[C*128, N] -> [128, C, N] contiguous."""
    c = a.shape[0] // 128
    return np.ascontiguousarray(a.reshape(c, 128, *a.shape[1:]).transpose(1, 0, 2))


def build_in_maps(inputs):
    x = np.asarray(inputs["x"], dtype=np.float32)
    pad = np.asarray(inputs["pad_mask"])
    wq_h = _chunked(np.asarray(inputs["wq"], dtype=np.float32)).astype(BF16NP)
    wk_h = _chunked(np.asarray(inputs["wk"], dtype=np.float32)).astype(BF16NP)
    wv_h = _chunked(np.asarray(inputs["wv"], dtype=np.float32)).astype(BF16NP)

    in_maps = []
    for c in range(8):
        b, h = divmod(c, 2)
        q0 = h * NQ
        xtb = _chunked(x[b].T).astype(BF16NP)               # [128, 8, 2048]
        xqb = _chunked(x[b, q0 : q0 + NQ, :].T).astype(BF16NP)  # [128, 8, 1024]
        keep = (~pad[b]).astype(np.float32)                     # [2048]
        padk = np.ascontiguousarray(keep.reshape(NKT, 128).T)   # [128, 16]
        thr = (
            np.add.outer(np.arange(128, dtype=np.float32),
                         128.0 * np.arange(NKT, dtype=np.float32))
            - np.float32(q0)
        ).astype(np.float32)                                    # [128, 16]
        in_maps.append({
            "xt": xtb, "xq": xqb, "wq": wq_h, "wk": wk_h, "wv": wv_h,
            "padk": padk, "thr": np.ascontiguousarray(thr),
        })
    return in_maps


def kernel(**inputs):
    global _NC_CACHE
    if _NC_CACHE is None:
        _NC_CACHE = _build_nc()
    nc = _NC_CACHE

    in_maps = build_in_maps(inputs)
    res = bass_utils.run_bass_kernel_spmd(nc, in_maps, core_ids=list(range(8)))
    out = np.stack(
        [np.concatenate([res.results[2 * b]["out"], res.results[2 * b + 1]["out"]], axis=0)
         for b in range(B)]
    )
    return out.astype(np.float32)


# revision 12
# speedup vs baseline: 1.9273x; 1.2176x over previous
"""Single-head causal attention (B=4, L=2048, D=1024) on 8 TRN2 NeuronCores.

Sharding: data-parallel over batch (4) x query-halves (2) = 8 cores.
Each core computes K/V projections over its batch's full 2048 keys and
Q/scores/AV over its 1024-query half.

Device-side layout (per core, all bf16 matmul operands, f32 PSUM accum):
  - scores computed TRANSPOSED: sT[k_tile(128 part), q(1024 free)] so the
    pad mask (a per-key quantity) is a per-partition tensor_scalar and the
    softmax normalizer Z comes from a ones-column appended to V (no
    partition reductions, no P transposes anywhere).
  - masked_fill semantics (exact, incl. degenerate all-masked rows):
      cmp[k,q]  = (iota_q >= thresh[k]) * padkeep[k]   in {0,1}
      sM        = (s_raw + 960) * cmp                  # 0 where masked
      E         = exp(sM/32 - 30)                      # = exp(s/32) kept,
                                                       #   exp(-30) masked
    All-masked rows get exactly-uniform weights, matching jax softmax of
    an all -10000 row.
  - out[q,m] = (E^T V)[q,m] / Z[q], Z from the ones column of V.
"""
import sys

if "/opt/trn_rl_repo" not in sys.path:
    sys.path.insert(0, "/opt/trn_rl_repo")

import numpy as np
import ml_dtypes

import concourse.bass as bass
import concourse.mybir as mybir
from concourse import bacc, tile
from concourse import bass_utils

F32 = mybir.dt.float32
BF16 = mybir.dt.bfloat16
BF16NP = ml_dtypes.bfloat16

B, L, D = 4, 2048, 1024
NQ = L // 2          # queries per core
NKT = L // 128       # 16 k-tiles
NMC = D // 128       # 8 contraction/model chunks
NQT = NQ // 128      # 8 q-tiles per core

_NC_CACHE = None


def _build_nc():
    nc = bacc.Bacc(None, target_bir_lowering=False)

    xt_d = nc.dram_tensor("xt", [128, NMC, L], BF16, kind="ExternalInput")
    xq_d = nc.dram_tensor("xq", [128, NMC, NQ], BF16, kind="ExternalInput")
    wq_d = nc.dram_tensor("wq", [128, NMC, D], BF16, kind="ExternalInput")
    wk_d = nc.dram_tensor("wk", [128, NMC, D], BF16, kind="ExternalInput")
    wv_d = nc.dram_tensor("wv", [128, NMC, D], BF16, kind="ExternalInput")
    padk_d = nc.dram_tensor("padk", [128, NKT], F32, kind="ExternalInput")
    thr_d = nc.dram_tensor("thr", [128, NKT], F32, kind="ExternalInput")
    out_d = nc.dram_tensor("out", [NQ, D], F32, kind="ExternalOutput")

    AL = mybir.AluOpType
    AF = mybir.ActivationFunctionType

    with tile.TileContext(nc) as tc:
        with (
            tc.tile_pool(name="c", bufs=1) as cpool,
            tc.tile_pool(name="sh", bufs=1) as spool,
            tc.tile_pool(name="wk_", bufs=3) as wpool,
            tc.tile_pool(name="pp", bufs=3, space="PSUM") as pp,
            tc.tile_pool(name="ppo", bufs=2, space="PSUM") as ppo,
            tc.tile_pool(name="ppz", bufs=1, space="PSUM") as ppz,
        ):
            # persistent tiles
            xt_sb = spool.tile([128, NMC, L], BF16, name="xt_sb", tag="big")
            wk_sb = spool.tile([128, NMC, D], BF16, name="wk_sb", tag="wkqt")
            wv_sb = cpool.tile([128, NMC, D], BF16, name="wv_sb")
            wq_sb = cpool.tile([128, NMC, D], BF16, name="wq_sb")
            xq_sb = cpool.tile([128, NMC, NQ], BF16, name="xq_sb")
            padk_sb = cpool.tile([128, NKT], F32, name="padk_sb")
            thr_sb = cpool.tile([128, NKT], F32, name="thr_sb")
            kT_sb = cpool.tile([128, NMC, L], BF16, name="kT_sb")
            v_sb = cpool.tile([128, NKT, D + 1], BF16, name="v_sb")
            iota_sb = cpool.tile([128, NQ], F32, name="iota_sb")
            bias_sb = cpool.tile([128, 1], F32, name="bias_sb")

            nc.sync.dma_start(xt_sb[:], xt_d[:])
            nc.sync.dma_start(wk_sb[:], wk_d[:])
            nc.sync.dma_start(wv_sb[:], wv_d[:])
            nc.sync.dma_start(wq_sb[:], wq_d[:])
            nc.sync.dma_start(xq_sb[:], xq_d[:])
            nc.sync.dma_start(padk_sb[:], padk_d[:])
            nc.sync.dma_start(thr_sb[:], thr_d[:])

            # local q column f (= 128*jl + fi) maps to global q-tile 2*jl + h;
            # iota encodes q_glob - 128*h = 256*jl + fi; thresh data absorbs h.
            nc.gpsimd.iota(
                out=iota_sb[:].rearrange("p (j f) -> p j f", f=128),
                pattern=[[256, NQT], [1, 128]], base=0, channel_multiplier=0,
                allow_small_or_imprecise_dtypes=True,
            )
            nc.vector.memset(bias_sb[:], -30.0)
            nc.vector.memset(v_sb[:, :, D : D + 1], 1.0)

            # ---- Phase 1a: kT[m, tok] = wk.T @ x ----
            for mi in range(NMC):
                for tb in range(L // 512):
                    ps = pp.tile([128, 512], F32, name="ps")
                    for d in range(NMC):
                        nc.tensor.matmul(
                            ps[:],
                            lhsT=wk_sb[:, d, mi * 128 : (mi + 1) * 128],
                            rhs=xt_sb[:, d, tb * 512 : (tb + 1) * 512],
                            start=(d == 0), stop=(d == NMC - 1),
                        )
                    nc.scalar.copy(kT_sb[:, mi, tb * 512 : (tb + 1) * 512], ps[:])

            # ---- Phase 1b: V[tok, m] = x @ wv  (lhsT = xT chunk) ----
            for kt in range(NKT):
                for mb in range(D // 512):
                    ps = pp.tile([128, 512], F32, name="ps")
                    for d in range(NMC):
                        nc.tensor.matmul(
                            ps[:],
                            lhsT=xt_sb[:, d, kt * 128 : (kt + 1) * 128],
                            rhs=wv_sb[:, d, mb * 512 : (mb + 1) * 512],
                            start=(d == 0), stop=(d == NMC - 1),
                        )
                    nc.scalar.copy(v_sb[:, kt, mb * 512 : (mb + 1) * 512], ps[:])

            # ---- Phase 1c: qT[m, q] = wq.T @ xq ----
            qT_sb = spool.tile([128, NMC, NQ], BF16, name="qT_sb", tag="wkqt")
            for mi in range(NMC):
                for qb in range(NQ // 512):
                    ps = pp.tile([128, 512], F32, name="ps")
                    for d in range(NMC):
                        nc.tensor.matmul(
                            ps[:],
                            lhsT=wq_sb[:, d, mi * 128 : (mi + 1) * 128],
                            rhs=xq_sb[:, d, qb * 512 : (qb + 1) * 512],
                            start=(d == 0), stop=(d == NMC - 1),
                        )
                    nc.scalar.copy(qT_sb[:, mi, qb * 512 : (qb + 1) * 512], ps[:])

            # ---- Phase 2: scores (transposed) + mask + exp, per k-tile ----
            # Local q-tile jl holds global q-tile 2*jl + h, so k-tile kt is
            # causally live only for jl >= ceil((kt-1)/2): a contiguous tail
            # of the local q axis. Fully-dead (kt, jl) pairs are skipped;
            # the h=0 core's extra tile per jl is killed by cmp data.
            E_sb = spool.tile([128, NKT, NQ], BF16, name="E_sb", tag="big")
            for kt in range(NKT):
                jl0 = kt // 2  # ceil((kt-1)/2): first local q-tile that sees kt
                f0 = jl0 * 128
                cmp = wpool.tile([128, NQ], F32, name="cmp", bufs=2)
                nc.vector.tensor_scalar(
                    out=cmp[:, f0:], in0=iota_sb[:, f0:],
                    scalar1=thr_sb[:, kt : kt + 1], scalar2=padk_sb[:, kt : kt + 1],
                    op0=AL.is_ge, op1=AL.mult,
                )
                s_sb = wpool.tile([128, NQ], F32, name="s_sb", bufs=3)
                blocks = []
                f = f0
                while f < NQ:
                    w = min(512, NQ - f)
                    blocks.append((f, w))
                    f += w
                for (fb, w) in blocks:
                    ps = pp.tile([128, 512], F32, name="ps")
                    for m in range(NMC):
                        nc.tensor.matmul(
                            ps[:, 0:w],
                            lhsT=kT_sb[:, m, kt * 128 : (kt + 1) * 128],
                            rhs=qT_sb[:, m, fb : fb + w],
                            start=(m == 0), stop=(m == NMC - 1),
                        )
                    nc.vector.scalar_tensor_tensor(
                        out=s_sb[:, fb : fb + w], in0=ps[:, 0:w],
                        scalar=960.0, in1=cmp[:, fb : fb + w],
                        op0=AL.add, op1=AL.mult,
                    )
                nc.scalar.activation(
                    out=E_sb[:, kt, f0:], in_=s_sb[:, f0:],
                    func=AF.Exp, bias=bias_sb[:], scale=0.03125,
                )

            # ---- Phase 3: out[q,m] = (E^T @ [V|1])[q,m] / Z[q] ----
            for jl in range(NQT):
                nkt = 2 * jl + 2  # causally-live k-tiles for this q-tile
                po = ppo.tile([128, D], F32, name="po")
                pz = ppz.tile([128, 1], F32, name="pz")
                for kt in range(nkt):
                    lhsT = E_sb[:, kt, jl * 128 : (jl + 1) * 128]
                    nc.tensor.matmul(po[:, 0:512], lhsT=lhsT, rhs=v_sb[:, kt, 0:512],
                                     start=(kt == 0), stop=(kt == nkt - 1))
                    nc.tensor.matmul(po[:, 512:1024], lhsT=lhsT, rhs=v_sb[:, kt, 512:1024],
                                     start=(kt == 0), stop=(kt == nkt - 1))
                    nc.tensor.matmul(pz[:], lhsT=lhsT, rhs=v_sb[:, kt, D : D + 1],
                                     start=(kt == 0), stop=(kt == nkt - 1))
                rec = wpool.tile([128, 1], F32, name="rec", bufs=2)
                nc.vector.reciprocal(rec[:], pz[:])
                o_sb = wpool.tile([128, D], F32, name="o_sb", bufs=3)
                nc.vector.tensor_scalar(
                    out=o_sb[:], in0=po[:], scalar1=rec[:], scalar2=None, op0=AL.mult,
                )
                nc.sync.dma_start(out_d[jl * 128 : (jl + 1) * 128, :], o_sb[:])

    nc.compile()
    return nc


def _chunked(a):
    """[C*128, N] -> [128, C, N] contiguous."""
    c = a.shape[0] // 128
    return np.ascontiguousarray(a.reshape(c, 128, *a.shape[1:]).transpose(1, 0, 2))


def _qsel(h):
    """Global query rows handled by half h: interleaved 128-row q-tiles."""
    return np.concatenate(
        [np.arange(128 * (2 * jl + h), 128 * (2 * jl + h) + 128) for jl in range(NQT)]
    )


def build_in_maps(inputs):
    x = np.asarray(inputs["x"], dtype=np.float32)
    pad = np.asarray(inputs["pad_mask"])
    wq_h = _chunked(np.asarray(inputs["wq"], dtype=np.float32)).astype(BF16NP)
    wk_h = _chunked(np.asarray(inputs["wk"], dtype=np.float32)).astype(BF16NP)
    wv_h = _chunked(np.asarray(inputs["wv"], dtype=np.float32)).astype(BF16NP)

    in_maps = []
    for c in range(8):
        b, h = divmod(c, 2)
        qsel = _qsel(h)
        xtb = _chunked(x[b].T).astype(BF16NP)               # [128, 8, 2048]
        xqb = _chunked(x[b, qsel, :].T).astype(BF16NP)      # [128, 8, 1024]
        keep = (~pad[b]).astype(np.float32)                     # [2048]
        padk = np.ascontiguousarray(keep.reshape(NKT, 128).T)   # [128, 16]
        # keep iff iota (= q_glob - 128h) >= thresh = 128*kt + p - 128*h
        thr = (
            np.add.outer(np.arange(128, dtype=np.float32),
                         128.0 * np.arange(NKT, dtype=np.float32))
            - np.float32(128 * h)
        ).astype(np.float32)                                    # [128, 16]
        in_maps.append({
            "xt": xtb, "xq": xqb, "wq": wq_h, "wk": wk_h, "wv": wv_h,
            "padk": padk, "thr": np.ascontiguousarray(thr),
        })
    return in_maps


def kernel(**inputs):
    global _NC_CACHE
    if _NC_CACHE is None:
        _NC_CACHE = _build_nc()
    nc = _NC_CACHE

    in_maps = build_in_maps(inputs)
    res = bass_utils.run_bass_kernel_spmd(nc, in_maps, core_ids=list(range(8)))
    out = np.empty((B, L, D), dtype=np.float32)
    for b in range(B):
        for h in range(2):
            out[b, _qsel(h)] = res.results[2 * b + h]["out"]
    return out
